# revision 15
# baseline (speedup 1.0000x reference)
"""Trainium2 Bass kernel for nn_AttentionPropagation.

Reference computation (per batch b):
  q = Wq@x1 + bq ; k = Wk@x2 + bk ; v = Wv@x2 + bv    (1x1 convs, [C, N])
  per head h (D=64): S = q_h^T k_h ; S = where(mask, S, -1e6)
  P = softmax(S / sqrt(D), axis=keys) ; attn = v_h @ P^T
  mh = Wmh@attn + bmh
  cat = [x1; mh] ; h = relu(BN(W1@cat + b1)) ; y = x1 + W2@h + b2

Sharding: 8 cores = (batch b in 0..3) x (query-half nh in 0..1).
Each core computes the full attention + MLP for its [C, 1024] query slice
against that batch's keys. Keys are compacted on the host (masked keys
dropped, padded to MPAD); padding columns get an exp bias of -125000 so
their softmax weight is exactly 0 (matching the reference's -1e6 fill,
which also underflows to 0 after exp in fp32).

Attention layout (per core): scores are computed TRANSPOSED, S^T[m, n],
so the softmax denominator is folded into the AV matmul as an extra
all-ones column of v^T; no on-chip transposes are needed anywhere.

Host-side folds (exact, float64):
  - BatchNorm folded into W1/b1.
  - bv folded downstream: normalized attn contributes +bv (sum of softmax
    weights = 1), so b1 += W1[:, C:] @ (Wmh@bv + bmh); kernel skips bv/bmh.
"""

import os
import sys

for _p in ("/opt/trn_rl_repo", "/root/.axon_site/_ro/trn_rl_repo"):
    if os.path.isdir(_p) and _p not in sys.path:
        sys.path.append(_p)

import ml_dtypes
import numpy as np

import concourse.bacc as bacc
import concourse.bass as bass
import concourse.mybir as mybir
import concourse.tile as tile
from concourse import bass_utils
from concourse.bass import ts

B, C, H, N, M = 4, 256, 4, 2048, 2048
D = C // H            # 64
NCORES = 8
NL = N // 2           # 1024 queries per core
MPAD = 1152           # padded (compacted) key count, multiple of 128
MC = MPAD // 128      # key chunks
BN_EPS = 1e-5
F32 = mybir.dt.float32
F32R = mybir.dt.float32r
BF16 = mybir.dt.bfloat16

# matmul-operand dtype: "bf16" (fast-weight-load + full PE rate) or "f32r"
# (tfloat32: ~5x lower error, slower weight loads)
MMDT_NAME = os.environ.get("KERNEL_MMDT", "bf16")
MMDT = {"bf16": BF16, "f32r": F32R}[MMDT_NAME]


def build_nc():
    nc = bacc.Bacc("TRN2", target_bir_lowering=False, debug=False)

    dram = {}
    def din(name, shape, dt=F32):
        dram[name] = nc.dram_tensor(name, shape, dt, kind="ExternalInput").ap()
    din("x1s", [C, NL], MMDT)
    din("x1r", [C, NL])
    din("x2c", [C, MPAD], MMDT)
    din("maskb", [128, MC])
    din("wqt", [C, C], MMDT)
    din("wkt", [C, C], MMDT)
    din("wvt", [C, C], MMDT)
    din("wmht", [C, C], MMDT)
    din("w1t", [2 * C, 2 * C], MMDT)
    din("w2t", [2 * C, C], MMDT)
    din("onesd", [128, H], MMDT)
    din("bq", [128, 2])
    din("bk", [128, 2])
    din("b1", [128, 4])
    din("b2", [128, 2])
    dram["y"] = nc.dram_tensor("y", [C, NL], F32, kind="ExternalOutput").ap()
    dram["dn"] = nc.dram_tensor("dn_bounce", [H, NL], F32).ap()

    with tile.TileContext(nc) as tc:
        build_kernel(tc, dram)
    nc.compile()
    return nc


def build_kernel(tc, dram):
    from contextlib import ExitStack
    nc = tc.nc
    ALU = mybir.AluOpType
    AF = mybir.ActivationFunctionType

    ctx = ExitStack()
    const = ctx.enter_context(tc.tile_pool(name="const", bufs=1))
    work = ctx.enter_context(tc.tile_pool(name="work", bufs=1))
    ptp = ctx.enter_context(tc.tile_pool(name="ptp", bufs=4))
    bcp = ctx.enter_context(tc.tile_pool(name="bcp", bufs=2))
    psum = ctx.enter_context(tc.tile_pool(name="psum", bufs=2, space="PSUM"))

    def mm(out, lhsT, rhs, start, stop):
        nc.tensor.matmul(out, lhsT, rhs, start=start, stop=stop)

    # ---- load inputs/weights into SBUF ----
    # spread the input loads across four engine sequencers: descriptor
    # generation is ~0.7us per dma_start and serializes per sequencer
    dma_engines = [nc.sync, nc.gpsimd, nc.scalar]
    dma_rr = [0]

    def next_eng():
        e = dma_engines[dma_rr[0] % len(dma_engines)]
        dma_rr[0] += 1
        return e

    def load_tiles(name, rows, width, nt):
        dt = dram[name].dtype
        big = const.tile([rows, nt, width], dt, tag=name, name=f"{name}_all")
        next_eng().dma_start(out=big,
                             in_=dram[name].rearrange("(b p) w -> p b w", p=rows))
        return [big[:, i, :] for i in range(nt)]

    def load_one(name, shape):
        t = const.tile(shape, F32, tag=name, name=f"{name}_sb")
        next_eng().dma_start(out=t, in_=dram[name])
        return t

    wkt_sb = load_tiles("wkt", 128, C, 2)
    wvt_sb = load_tiles("wvt", 128, C, 2)
    maskb_sb = load_one("maskb", [128, MC])
    bq_sb = load_one("bq", [128, 2])
    bk_sb = load_one("bk", [128, 2])
    x2_sb = load_tiles("x2c", 128, MPAD, 2)
    wqt_sb = load_tiles("wqt", 128, C, 2)
    x1_sb = load_tiles("x1s", 128, NL, 2)
    b1_sb = load_one("b1", [128, 4])
    b2_sb = load_one("b2", [128, 2])
    wmh_sb = load_tiles("wmht", D, C, 4)       # per-head rows of Wmh^T
    w1t_sb = load_tiles("w1t", 128, 2 * C, 4)
    w2t_sb = load_tiles("w2t", 128, C, 4)
    x1r_sb = load_tiles("x1r", 128, NL, 2)

    # ---- q projection: q[cb] = [128 co, NL] ----
    q_sb = []
    for cb in range(2):
        ps = psum.tile([128, NL], F32, tag="st", name=f"q_ps{cb}")
        for kc in range(2):
            for nf in range(2):
                mm(ps[:, ts(nf, 512)], wqt_sb[kc][:, ts(cb, 128)],
                   x1_sb[kc][:, ts(nf, 512)], start=(kc == 0), stop=(kc == 1))
        qt = work.tile([128, NL], MMDT, tag=f"q{cb}", name=f"q{cb}")
        nc.vector.tensor_scalar_add(qt, ps, bq_sb[:, cb:cb + 1])
        q_sb.append(qt)

    # ---- k projection: k[cb] = [128 co, MPAD] ----
    k_sb = []
    kchunks = [(0, 512), (512, 512), (1024, MPAD - 1024)]
    for cb in range(2):
        kt = work.tile([128, MPAD], MMDT, tag=f"k{cb}", name=f"k{cb}")
        for (off, w) in kchunks:
            ps = psum.tile([128, 512], F32, tag="st", name=f"k_ps{cb}_{off}")
            for kc in range(2):
                mm(ps[:, 0:w], wkt_sb[kc][:, ts(cb, 128)],
                   x2_sb[kc][:, off:off + w], start=(kc == 0), stop=(kc == 1))
            nc.vector.tensor_scalar_add(kt[:, off:off + w], ps[:, 0:w],
                                        bk_sb[:, cb:cb + 1])
        k_sb.append(kt)

    # ---- v^T: vt[mc] = [128 m, 4*(D+1)]; per head D cols + ones col ----
    vt_sb = []
    for mc in range(MC):
        ps = psum.tile([128, C], F32, tag="st", name=f"v_ps{mc}")
        for kc in range(2):
            mm(ps, x2_sb[kc][:, ts(mc, 128)], wvt_sb[kc],
               start=(kc == 0), stop=(kc == 1))
        vt = work.tile([128, H * (D + 1)], MMDT, tag=f"vt{mc}", name=f"vt{mc}")
        vt_r = vt.rearrange("p (h x) -> p h x", x=D + 1)
        nc.gpsimd.dma_start(out=vt_r[:, :, D:D + 1], in_=dram["onesd"])
        nc.vector.tensor_copy(out=vt_r[:, :, 0:D],
                              in_=ps.rearrange("p (h x) -> p h x", x=D))
        vt_sb.append(vt)

    # ---- attention per head: S^T = k_h^T q_h, P = exp(S^T/8 + maskbias),
    #      av[0:D] = v P, av[D] = softmax denominator ----
    araw = [work.tile([D + 1, NL], F32, tag=f"araw{h}", name=f"araw{h}")
            for h in range(H)]
    attn = []
    den = []

    for h in range(H):
        cb, off = h // 2, (h % 2) * D
        av = psum.tile([D + 1, NL], F32, tag="av", name=f"av{h}")
        for mc in range(MC):
            st = psum.tile([128, NL], F32, tag="st", name=f"st{h}_{mc}")
            for nf in range(2):
                mm(st[:, ts(nf, 512)],
                   k_sb[cb][off:off + D, ts(mc, 128)],
                   q_sb[cb][off:off + D, ts(nf, 512)], start=True, stop=True)
            pt = ptp.tile([128, NL], MMDT, tag="pt", name=f"pt{h}_{mc}")
            nc.scalar.activation(out=pt, in_=st, func=AF.Exp,
                                 bias=maskb_sb[:, mc:mc + 1], scale=0.125)
            for nf in range(2):
                mm(av[:, ts(nf, 512)],
                   vt_sb[mc][:, h * (D + 1):(h + 1) * (D + 1)],
                   pt[:, ts(nf, 512)], start=(mc == 0), stop=(mc == MC - 1))
        nc.vector.tensor_copy(out=araw[h], in_=av)
        # normalization, pipelined per head-PAIR. The denominator row is
        # DMA-scattered across 64 partitions so the exact reciprocal runs
        # 16 elems/lane (~0.2us) instead of 1024 on one lane (~6.5us);
        # the result bounces through DRAM for the partition-broadcast read.
        pair = h // 2
        if h % 2 == 0:
            den.append(work.tile([64, 32], F32, tag=f"den{pair}",
                                 name=f"den{pair}"))
        nc.sync.dma_start(
            out=den[pair].rearrange("p (e j) -> p e j", j=16)[:, h % 2, :],
            in_=araw[h][D:D + 1, :])
        if h % 2 == 1:
            rcp = work.tile([64, 32], F32, tag=f"rcp{pair}", name=f"rcp{pair}")
            nc.vector.reciprocal(out=rcp, in_=den[pair])
            dnt = dram["dn"]
            dn_scat = bass.AP(tensor=dnt.tensor, offset=2 * pair * NL,
                              ap=[[16, 64], [NL, 2], [1, 16]])
            nc.sync.dma_start(out=dn_scat,
                              in_=rcp.rearrange("p (e j) -> p e j", j=16))
            for hh in (h - 1, h):
                bc = bcp.tile([D, NL], F32, tag="bc", name=f"bc{hh}")
                dnr = dram["dn"][hh:hh + 1, :]
                bcast_ap = bass.AP(tensor=dnr.tensor, offset=dnr.offset,
                                   ap=[[0, D]] + list(dnr.ap[1:]))
                nc.sync.dma_start(out=bc, in_=bcast_ap)
                at = work.tile([D, NL], MMDT, tag=f"attn{hh}", name=f"attn{hh}")
                nc.vector.tensor_mul(out=at, in0=araw[hh][0:D, :], in1=bc)
                attn.append(at)

    # ---- mh = Wmh^T.T @ attn (4 K=64 chunks; bmh/bv folded into b1) ----
    mh_sb = []
    for cb in range(2):
        ps = psum.tile([128, NL], F32, tag="st", name=f"mh_ps{cb}")
        for hc in range(4):
            for nf in range(2):
                mm(ps[:, ts(nf, 512)], wmh_sb[hc][:, ts(cb, 128)],
                   attn[hc][:, ts(nf, 512)], start=(hc == 0), stop=(hc == 3))
        mt = work.tile([128, NL], MMDT, tag=f"mh{cb}", name=f"mh{cb}")
        nc.vector.tensor_copy(out=mt, in_=ps)
        mh_sb.append(mt)

    # ---- h1 = relu(W1^T.T @ [x1; mh] + b1) ----
    cat_sb = [x1_sb[0], x1_sb[1], mh_sb[0], mh_sb[1]]
    h1_sb = []
    for ob in range(4):
        ps = psum.tile([128, NL], F32, tag="st", name=f"h1_ps{ob}")
        for kc in range(4):
            for nf in range(2):
                mm(ps[:, ts(nf, 512)], w1t_sb[kc][:, ts(ob, 128)],
                   cat_sb[kc][:, ts(nf, 512)], start=(kc == 0), stop=(kc == 3))
        ht = work.tile([128, NL], MMDT, tag=f"h1{ob}", name=f"h1{ob}")
        nc.vector.tensor_scalar(out=ht, in0=ps, scalar1=b1_sb[:, ob:ob + 1],
                                scalar2=0.0, op0=ALU.add, op1=ALU.max)
        h1_sb.append(ht)

    # ---- y = x1 + W2^T.T @ h1 + b2 ----
    for cb in range(2):
        ps = psum.tile([128, NL], F32, tag="st", name=f"y_ps{cb}")
        for kc in range(4):
            for nf in range(2):
                mm(ps[:, ts(nf, 512)], w2t_sb[kc][:, ts(cb, 128)],
                   h1_sb[kc][:, ts(nf, 512)], start=(kc == 0), stop=(kc == 3))
        yt = work.tile([128, NL], F32, tag=f"y{cb}", name=f"y{cb}")
        nc.vector.tensor_scalar_add(yt, ps, b2_sb[:, cb:cb + 1])
        nc.vector.tensor_add(out=yt, in0=yt, in1=x1r_sb[cb])
        nc.sync.dma_start(out=dram["y"][ts(cb, 128), :], in_=yt)

    ctx.close()


# ---------------------------------------------------------------------------
# host side
# ---------------------------------------------------------------------------

_NC_CACHE = {}


def _get_nc():
    if "nc" not in _NC_CACHE:
        _NC_CACHE["nc"] = build_nc()
    return _NC_CACHE["nc"]


def kernel(x1, x2, kv_mask, Wq, bq, Wk, bk, Wv, bv, Wmh, bmh,
           W1, b1, bn_gamma, bn_beta, bn_mean, bn_var, W2, b2):
    x1 = np.asarray(x1, np.float32)
    x2 = np.asarray(x2, np.float32)
    kv_mask = np.asarray(kv_mask).astype(bool)
    Wq, Wk, Wv, Wmh = (np.asarray(a, np.float32) for a in (Wq, Wk, Wv, Wmh))
    W1, W2 = np.asarray(W1, np.float32), np.asarray(W2, np.float32)
    bqv, bkv, bvv, bmhv = (np.asarray(a, np.float64) for a in (bq, bk, bv, bmh))
    b1v, b2v = np.asarray(b1, np.float64), np.asarray(b2, np.float64)
    g, bt = np.asarray(bn_gamma, np.float64), np.asarray(bn_beta, np.float64)
    mu, var = np.asarray(bn_mean, np.float64), np.asarray(bn_var, np.float64)

    # fold BN into W1/b1; fold bv/bmh into b1 (exact, float64)
    s = g / np.sqrt(var + BN_EPS)
    W1f = s[:, None] * W1.astype(np.float64)
    b1f = s * (b1v - mu) + bt
    b1f = b1f + W1f[:, C:] @ (np.asarray(Wmh, np.float64) @ bvv + bmhv)
    W1f32 = W1f.astype(np.float32)

    mmnp = {"bf16": ml_dtypes.bfloat16, "f32r": np.float32}[MMDT_NAME]
    shared = {
        "wqt": np.ascontiguousarray(Wq.T).astype(mmnp),
        "wkt": np.ascontiguousarray(Wk.T).astype(mmnp),
        "wvt": np.ascontiguousarray(Wv.T).astype(mmnp),
        "wmht": np.ascontiguousarray(Wmh.T).astype(mmnp),
        "w1t": np.ascontiguousarray(W1f32.T).astype(mmnp),
        "w2t": np.ascontiguousarray(W2.T).astype(mmnp),
        "bq": np.ascontiguousarray(bqv.astype(np.float32).reshape(2, 128).T),
        "bk": np.ascontiguousarray(bkv.astype(np.float32).reshape(2, 128).T),
        "b1": np.ascontiguousarray(b1f.astype(np.float32).reshape(4, 128).T),
        "b2": np.ascontiguousarray(b2v.astype(np.float32).reshape(2, 128).T),
        "onesd": np.ones((128, H), mmnp),
    }

    in_maps = []
    for core in range(NCORES):
        b, nh = core // 2, core % 2
        idx = np.nonzero(kv_mask[b])[0]
        mb = len(idx)
        assert mb <= MPAD, f"batch {b}: {mb} unmasked keys > MPAD={MPAD}"
        x2c = np.zeros((C, MPAD), np.float32)
        x2c[:, :mb] = x2[b][:, idx]
        mbias = np.full(MPAD, -125000.0, np.float32)
        mbias[:mb] = 0.0
        im = dict(shared)
        x1slice = np.ascontiguousarray(x1[b][:, nh * NL:(nh + 1) * NL])
        im["x1s"] = x1slice.astype(mmnp)
        im["x1r"] = x1slice
        im["x2c"] = x2c.astype(mmnp)
        im["maskb"] = np.ascontiguousarray(mbias.reshape(MC, 128).T)
        in_maps.append(im)

    nc = _get_nc()
    res = bass_utils.run_bass_kernel_spmd(nc, in_maps, core_ids=list(range(NCORES)))
    _NC_CACHE["last_res"] = res

    out = np.empty((B, C, N), np.float32)
    for core in range(NCORES):
        b, nh = core // 2, core % 2
        out[b][:, nh * NL:(nh + 1) * NL] = res.results[core]["y"]
    return out


if __name__ == "__main__":
    build_nc()
    print("built + compiled OK")
